# revision 1
# baseline (speedup 1.0000x reference)
"""ConvNeXt block (dwconv7 -> LN -> pwconv1 -> GELU -> GRN -> pwconv2 -> residual)
for Trainium2, batch-parallel across 8 NeuronCores (2 batches per core).

Self-contained: hardcodes shapes B=16, C=512, T=2048, I=1536, K=7.

Math folding (host-side, weight-sized only):
  LN:  y_ln = (y - mu_t) * A_t * ln_g + ln_b      (A_t = rsqrt(var_t + eps))
  mm1: h_pre[i,t] = A_t * sum_c w1p[i,c] y[c,t] + B_t * W1s[i] + b1p[i]
       with w1p = w1 * ln_g,  W1s[i] = sum_c w1p[i,c],
       b1p = b1 + w1 @ ln_b,  B_t = -A_t * mu_t
       (A_t applied to rhs before matmul; W1s x B via K=1 rank-1 matmul into
        PSUM; b1p via GELU activation bias)
  GRN: h' = h * ss[i] + grn_b[i],  ss = 1 + grn_g * gx * d,
       gx = sqrt(sum_t h^2), d = 1/(mean_i gx + eps)
  mm2: out = (w2 * ss).T-contracted with h + (b2 + w2 @ grn_b) + residual
"""
import sys

sys.path.insert(0, "/opt/trn_rl_repo")

import numpy as np
import concourse.bacc as bacc
import concourse.tile as tile
from concourse import mybir
from concourse.bass_utils import run_bass_kernel_spmd

F32 = mybir.dt.float32
F32R = mybir.dt.float32r
F16 = mybir.dt.float16
AF = mybir.ActivationFunctionType
OP = mybir.AluOpType

B, C, T, I, K = 16, 512, 2048, 1536, 7
NCORES = 8
BPC = B // NCORES          # batches per core
CC = C // 128              # 4 c-chunks
IC = I // 128              # 12 i-chunks
TC = T // 512              # 4 t-chunks
TN = 512                   # matmul free-dim tile
LN_EPS = 1e-6
GRN_EPS = 1e-6

_CACHE = {}


def _build(trace_sim=False, reps=1):
    nc = bacc.Bacc("TRN2", target_bir_lowering=False, debug=False,
                   num_devices=NCORES)
    dram = {}

    def din(name, shape, dt=F32):
        dram[name] = nc.dram_tensor(name, shape, dt, kind="ExternalInput").ap()
        return dram[name]

    x_d = din("x", [BPC, C, T])                      # per-core batches
    w1pT_d = din("w1pT", [C, I], F32R)               # (w1*ln_g).T  [c, i]
    w1s_d = din("w1s", [1, I], F32R)                 # row: sum_c w1p[i,c]
    b1p_d = din("b1p", [128, IC])                    # col-chunked b1p
    w2T_d = din("w2T", [I, C], F16)                 # w2.T  [i, c]
    b2p_d = din("b2p", [128, CC])                    # col-chunked b2 + w2@grn_b
    grng_d = din("grng", [128, IC])                  # col-chunked grn_g
    dww_d = din("dww", [128, CC, K])                 # depthwise taps per c-chunk
    dwb_d = din("dwb", [128, CC])                    # depthwise bias per c-chunk
    out_d = nc.dram_tensor("out", [BPC, C, T], F32, kind="ExternalOutput").ap()

    with tile.TileContext(nc, trace_sim=trace_sim) as tc:
        for _ in range(reps):
            _kernel_body(nc, tc, x_d, w1pT_d, w1s_d, b1p_d, w2T_d, b2p_d,
                         grng_d, dww_d, dwb_d, out_d)
    nc.compile()
    return nc


def _kernel_body(nc, tc, x_d, w1pT_d, w1s_d, b1p_d, w2T_d, b2p_d,
                 grng_d, dww_d, dwb_d, out_d):
    from contextlib import ExitStack
    HB = 2            # t-halves for conv/stats interleave
    HT = T // HB      # 1024
    G6 = IC // 2      # h staged/streamed in 6-i-chunk groups
    ctx = ExitStack()
    with ctx:
        ctx.enter_context(nc.allow_low_precision(
            reason="f32r matmul operand rounding is intentional"))
        singles = ctx.enter_context(tc.tile_pool(name="singles", bufs=1))
        xp = ctx.enter_context(tc.tile_pool(name="xp", bufs=2))
        xrp = ctx.enter_context(tc.tile_pool(name="xrp", bufs=2))
        t06p = ctx.enter_context(tc.tile_pool(name="t06p", bufs=1))
        yp = ctx.enter_context(tc.tile_pool(name="yp", bufs=3))
        ysqp = ctx.enter_context(tc.tile_pool(name="ysqp", bufs=3))
        rowp = ctx.enter_context(tc.tile_pool(name="rowp", bufs=4))
        abcsb = ctx.enter_context(tc.tile_pool(name="abcsb", bufs=2))
        yscp = ctx.enter_context(tc.tile_pool(name="yscp", bufs=3))
        hstp = ctx.enter_context(tc.tile_pool(name="hstp", bufs=3))
        hrdp = ctx.enter_context(tc.tile_pool(name="hrdp", bufs=2))
        sqp = ctx.enter_context(tc.tile_pool(name="sqp", bufs=1))
        gxp = ctx.enter_context(tc.tile_pool(name="gxp", bufs=2))
        w2p = ctx.enter_context(tc.tile_pool(name="w2p", bufs=1))
        op_ = ctx.enter_context(tc.tile_pool(name="op", bufs=2))
        # PSUM pools: 6 shared matmul banks + 2 stats banks
        mmps = ctx.enter_context(tc.tile_pool(name="mmps", bufs=6, space="PSUM"))
        mm1ps = mmps
        mm2ps = mmps
        smps = ctx.enter_context(tc.tile_pool(name="smps", bufs=2, space="PSUM"))
        hdram = ctx.enter_context(tc.tile_pool(name="hdram", bufs=2, space="DRAM"))

        # ---- constants (small ones first so conv starts immediately) ----
        dww = singles.tile([128, CC, K], F32)
        nc.gpsimd.dma_start(dww[:], dww_d)
        dwb = singles.tile([128, CC], F32)
        nc.gpsimd.dma_start(dwb[:], dwb_d)
        b1p = singles.tile([128, IC], F32)
        nc.gpsimd.dma_start(b1p[:], b1p_d)
        b2p = singles.tile([128, CC], F32)
        nc.gpsimd.dma_start(b2p[:], b2p_d)
        grng = singles.tile([128, IC], F32)
        nc.gpsimd.dma_start(grng[:], grng_d)
        w1s = singles.tile([1, I], F32R)
        nc.gpsimd.dma_start(w1s[:], w1s_d)
        w1pT = singles.tile([128, CC, I], F32R)
        nc.sync.dma_start(w1pT[:], w1pT_d.rearrange("(cc p) i -> p cc i", p=128))

        onesf = singles.tile([128, 1], F32)
        nc.vector.memset(onesf[:], 1.0)
        ones_col = singles.tile([128, 1], F32R)   # stats lhsT (K=128, M=1)
        nc.vector.tensor_copy(ones_col[:], onesf[:])
        onesrf = singles.tile([1, 128], F32)
        nc.vector.memset(onesrf[:], 1.0)
        ones_row = singles.tile([1, 128], F32R)   # bcast lhsT (K=1, M=128)
        nc.vector.tensor_copy(ones_row[:], onesrf[:])
        eps_ln = singles.tile([1, 1], F32)
        nc.vector.memset(eps_ln[:], LN_EPS)

        xv = x_d.rearrange("b (cc p) t -> b p cc t", p=128)
        def batch_slices(b):
            return [(q * TN, TN) for q in range(TC)]
        SLMAX = max(sl for b in range(BPC) for _, sl in batch_slices(b))
        for b in range(BPC):
            h_dr = hdram.tile([IC, 128, T], F16)
            w2t = w2p.tile([128, IC, C], F16, tag="w2t")
            nc.sync.dma_start(w2t[:], w2T_d.rearrange("(ic p) c -> p ic c", p=128))
            gxpart = gxp.tile([128, IC, TC], F32, tag="gxpart")
            slices = batch_slices(b)
            for (t0g, SL) in slices:
                lo_x = max(0, t0g - 3)
                hi_x = min(T, t0g + SL + 3)
                xn = hi_x - lo_x
                off = t0g - lo_x
                # ---- conv for this slice ----
                y_h = yp.tile([128, CC, SL], F32R, tag="y",
                              padded_shape=[128, CC, SLMAX])
                for ci in range(CC):
                    x_h = xp.tile([128, SL + 6], F32, tag="x",
                                  padded_shape=[128, SLMAX + 6])
                    nc.gpsimd.dma_start(x_h[:, 0:xn], xv[b, :, ci, lo_x:hi_x])
                    acc = y_h[:, ci, :]
                    # tap -3 on GPSIMD into a zero-padded temp
                    tmp06 = t06p.tile([128, SL], F32, tag="t06",
                                      padded_shape=[128, SLMAX])
                    d = -3
                    lo_l = max(0, lo_x - (t0g + d))
                    hi_l = min(SL, hi_x - (t0g + d))
                    s0 = lo_l + off + d
                    if lo_l > 0 or hi_l < SL:
                        nc.gpsimd.memset(tmp06[:], 0.0)
                    nc.gpsimd.tensor_scalar(
                        tmp06[:, lo_l:hi_l], x_h[:, s0:s0 + (hi_l - lo_l)],
                        dww[:, ci, 0:1], None, OP.mult)
                    nc.vector.tensor_scalar(acc, x_h[:, off:off + SL],
                                            dww[:, ci, 3:4], dwb[:, ci:ci + 1],
                                            OP.mult, OP.add)
                    for k in (2, 4, 1, 5, 6):
                        d = k - 3
                        lo_l = max(0, lo_x - (t0g + d))
                        hi_l = min(SL, hi_x - (t0g + d))
                        s0 = lo_l + off + d
                        nc.vector.scalar_tensor_tensor(
                            acc[:, lo_l:hi_l], x_h[:, s0:s0 + (hi_l - lo_l)],
                            dww[:, ci, k:k + 1], acc[:, lo_l:hi_l],
                            OP.mult, OP.add)
                    nc.vector.tensor_add(acc, acc, tmp06[:])

                # ---- stats + mm1 + GELU for the t-chunks of this slice ----
                for t in range(t0g // TN, (t0g + SL) // TN):
                    tl = slice(t * TN - t0g, (t + 1) * TN - t0g)
                    ts_ = slice(t * TN, (t + 1) * TN)
                    sumy = smps.tile([1, TN], F32, tag="smps")
                    sumsq = smps.tile([1, TN], F32, tag="smps")
                    for ci in range(CC):
                        ysq = ysqp.tile([128, TN], F32R)
                        nc.scalar.activation(ysq[:], y_h[:, ci, tl], AF.Square)
                        nc.tensor.matmul(sumy[:], ones_col[:], y_h[:, ci, tl],
                                         start=(ci == 0), stop=(ci == CC - 1))
                        nc.tensor.matmul(sumsq[:], ones_col[:], ysq[:],
                                         start=(ci == 0), stop=(ci == CC - 1))
                    mu = rowp.tile([1, TN], F32, tag="rowp")
                    nc.vector.tensor_scalar(mu[:], sumy[:], 1.0 / C, None, OP.mult)
                    var = rowp.tile([1, TN], F32, tag="rowp")
                    msq = rowp.tile([1, TN], F32, tag="rowp")
                    nc.vector.tensor_mul(msq[:], mu[:], mu[:])
                    nc.vector.scalar_tensor_tensor(var[:], sumsq[:], 1.0 / C,
                                                   msq[:], OP.mult, OP.subtract)
                    stdv = rowp.tile([1, TN], F32, tag="rowp")
                    nc.scalar.activation(stdv[:], var[:], AF.Sqrt, bias=eps_ln[:])
                    A_row = rowp.tile([1, TN], F32R, tag="rowp")
                    nc.vector.reciprocal(A_row[:], stdv[:])
                    B_row = rowp.tile([1, TN], F32R, tag="rowp")
                    nc.vector.scalar_tensor_tensor(B_row[:], mu[:], -1.0,
                                                   A_row[:].bitcast(F32),
                                                   OP.mult, OP.mult)
                    # broadcast A across partitions (K=1 matmul via mm1 slot)
                    abc_ps = mm1ps.tile([128, TN], F32, tag="mm")
                    nc.tensor.matmul(abc_ps[:], ones_row[:], A_row[:],
                                     start=True, stop=True)
                    abc = abcsb.tile([128, TN], F32)
                    nc.vector.tensor_copy(abc[:], abc_ps[:])
                    ysc = yscp.tile([128, CC, TN], F32R)
                    for ci in range(CC):
                        nc.gpsimd.tensor_mul(ysc[:, ci, :],
                                             y_h[:, ci, tl].bitcast(F32), abc[:])
                    for g in range(2):
                        hst = hstp.tile([128, G6, TN], F16, tag="hst")
                        for i6 in range(G6):
                            ii = g * G6 + i6
                            ph = mm1ps.tile([128, TN], F32, tag="mm")
                            isl = slice(ii * 128, (ii + 1) * 128)
                            for ci in range(CC):
                                nc.tensor.matmul(ph[:], w1pT[:, ci, isl],
                                                 ysc[:, ci, :],
                                                 start=(ci == 0), stop=False)
                            nc.tensor.matmul(ph[:], w1s[:, isl], B_row[:],
                                             start=False, stop=True)
                            nc.scalar.activation(hst[:, i6, :], ph[:], AF.Gelu,
                                                 bias=b1p[:, ii:ii + 1])
                            sq = sqp.tile([128, TN], F32, tag="sq")
                            if b % 2 == 0:
                                nc.scalar.activation(
                                    sq[:], hst[:, i6, :], AF.Square,
                                    accum_out=gxpart[:, ii, t:t + 1])
                            else:
                                nc.vector.scalar_tensor_tensor(
                                    sq[:], hst[:, i6, :], 1.0,
                                    hst[:, i6, :], OP.bypass, OP.mult,
                                    accum_out=gxpart[:, ii, t:t + 1])
                        nc.sync.dma_start(
                            h_dr[g * G6:(g + 1) * G6, :, ts_]
                            .rearrange("ii p t -> p ii t"), hst[:])

            # ---- GRN scale factors ----
            gxsq = gxp.tile([128, IC], F32, tag="gx2")
            nc.vector.tensor_reduce(gxsq[:], gxpart[:], axis=mybir.AxisListType.X,
                                    op=OP.add)
            gx = gxp.tile([128, IC], F32R, tag="gx2")
            nc.scalar.activation(gx[:], gxsq[:], AF.Sqrt)
            gsum = smps.tile([1, IC], F32, tag="smps")
            nc.tensor.matmul(gsum[:], ones_col[:], gx[:], start=True, stop=True)
            gtot = gxp.tile([1, 1], F32, tag="gx3")
            nc.vector.tensor_reduce(gtot[:], gsum[:], axis=mybir.AxisListType.X,
                                    op=OP.add)
            dinv = gxp.tile([1, 1], F32, tag="gx3")
            nc.vector.tensor_scalar(dinv[:], gtot[:], 1.0 / I, GRN_EPS,
                                    OP.mult, OP.add)
            d_row = gxp.tile([1, 1], F32R, tag="gx3")
            nc.vector.reciprocal(d_row[:], dinv[:])
            dbc = gxp.tile([128, 1], F32, tag="gx4")
            nc.gpsimd.partition_broadcast(dbc[:], d_row[:].bitcast(F32))
            ss = gxp.tile([128, IC], F32, tag="gx4")
            nc.vector.scalar_tensor_tensor(ss[:], gx[:].bitcast(F32), dbc[:],
                                           grng[:], OP.mult, OP.mult)
            nc.vector.tensor_scalar(ss[:], ss[:], 1.0, None, OP.add)
            for ii in range(IC):
                nc.vector.tensor_scalar(w2t[:, ii, :], w2t[:, ii, :],
                                        ss[:, ii:ii + 1], None, OP.mult)

            # ---- mm2 + bias + residual ----
            for t in range(TC):
                ts_ = slice(t * TN, (t + 1) * TN)
                po = [mm2ps.tile([128, TN], F32, tag="mm", name=f"po{ci}")
                      for ci in range(CC)]
                for g in range(2):
                    hrd = hrdp.tile([128, G6, TN], F16, tag="hrd")
                    nc.sync.dma_start(
                        hrd[:], h_dr[g * G6:(g + 1) * G6, :, ts_]
                        .rearrange("ii p t -> p ii t"))
                    for ci in range(CC):
                        csl = slice(ci * 128, (ci + 1) * 128)
                        for i6 in range(G6):
                            ii = g * G6 + i6
                            nc.tensor.matmul(po[ci][:], w2t[:, ii, csl],
                                             hrd[:, i6, :],
                                             start=(ii == 0), stop=(ii == IC - 1))
                for ci in range(CC):
                    o_sb = op_.tile([128, TN], F32)
                    nc.scalar.activation(o_sb[:], po[ci][:], AF.Identity,
                                         bias=b2p[:, ci:ci + 1])
                    x_res = xrp.tile([128, TN], F32)
                    nc.gpsimd.dma_start(x_res[:], xv[b, :, ci, ts_])
                    nc.gpsimd.tensor_add(o_sb[:], o_sb[:], x_res[:])
                    nc.gpsimd.dma_start(
                        out_d[b, ci * 128:(ci + 1) * 128, ts_], o_sb[:])


def _host_prep(inputs):
    w1 = inputs["w1"].astype(np.float64)
    ln_g = inputs["ln_g"].astype(np.float64)
    ln_b = inputs["ln_b"].astype(np.float64)
    w2 = inputs["w2"].astype(np.float64)
    w1p = w1 * ln_g[None, :]                         # [I, C]
    prep = {
        "w1pT": np.ascontiguousarray(w1p.T).astype(np.float32),
        "w1s": w1p.sum(axis=1)[None, :].astype(np.float32),
        "b1p": (inputs["b1"].astype(np.float64) + w1 @ ln_b)
               .astype(np.float32).reshape(IC, 128).T.copy(),
        "w2T": np.ascontiguousarray(w2.T).astype(np.float16),
        "b2p": (inputs["b2"].astype(np.float64)
                + w2 @ inputs["grn_b"].astype(np.float64))
               .astype(np.float32).reshape(CC, 128).T.copy(),
        "grng": inputs["grn_g"].reshape(IC, 128).T.copy().astype(np.float32),
        "dww": inputs["dw_w"].reshape(C, K).reshape(CC, 128, K)
               .transpose(1, 0, 2).copy().astype(np.float32),
        "dwb": inputs["dw_b"].reshape(CC, 128).T.copy().astype(np.float32),
    }
    return prep


def run(inputs, trace=False, **kw):
    if "nc" not in _CACHE:
        _CACHE["nc"] = _build()
    nc = _CACHE["nc"]
    prep = _host_prep(inputs)
    x = np.asarray(inputs["x"], dtype=np.float32)
    in_maps = []
    for c in range(NCORES):
        m = dict(prep)
        m["x"] = np.ascontiguousarray(x[c * BPC:(c + 1) * BPC])
        in_maps.append(m)
    res = run_bass_kernel_spmd(nc, in_maps, core_ids=list(range(NCORES)),
                               trace=trace, **kw)
    out = np.concatenate([r["out"] for r in res.results], axis=0)
    return out, res


def kernel(**inputs):
    out, _ = run(inputs)
    return out

